# revision 9
# baseline (speedup 1.0000x reference)
"""Trainium2 Bass kernel for nn_Blur (upfirdn2d: up=2, pad=(2,1,2,1), 4-tap
separable filter [1,3,3,1] x [1,3,3,1] / 64).

Input  x [16, 128, 128, 128] f32  ->  Output [16, 128, 256, 256] f32.

Math (polyphase decomposition of the zero-insertion upsample + conv):
  per axis, even outputs:  y[2i]   = (1*x[i-1] + 3*x[i]) / 8
            odd  outputs:  y[2i+1] = (3*x[i]   + 1*x[i+1]) / 8
Separable 2D:
  pass 1 (vertical, on TensorE): V = A.T @ X with A the banded [128, 256]
     polyphase matrix carrying the full 1/64 scale, in float32r (single
     full-speed matmul; rel err ~1e-4, far inside the 2e-2 gate).
     With ROWPAIR_LOAD the contraction is split into two accumulating
     K=64 matmuls over even/odd input rows so each SBUF partition can
     hold TWO consecutive DRAM rows -> 1KB input DMA descriptors
     (vs 512B), halving input packet count and engine overhead.
  pass 2 (horizontal): ScalarE computes u = 3V into SBUF (DVE may read
     at most one PSUM operand per instruction - NCC_IBVF027), then DVE
     adds  out[2j+k] = u[j] + V[j-1+2k]  read u(SBUF) + v(PSUM).

This kernel is DMA-bound: 84 MB/core over 16 DMA engines at ~22.5 GB/s
each ~= 242 us floor (235 us with 1KB input packets). Everything else is
sized to stay below that: balanced HWDGE queues (loads+stores alternate
between the SP and ACT rings), input loads issued LEAD groups ahead.

Sharding: pure data parallel, 2 examples per core x 8 cores. Each core
processes 256 channel-images of [128,128] in groups of 4 (matmul free dim
512).
"""

import numpy as np

H = 128
W = 128
N_CORES = 8
EX_PER_CORE = 2
NIMG_PER_CORE = EX_PER_CORE * 128  # 256 channel-images
GROUP = 4

# 2 input rows per SBUF partition (1KB DMA descriptors, K=64 x2 matmuls)
ROWPAIR_LOAD = False
# single fused DVE op per (eo, half) with k-interleaved contiguous writes
# (needs a zero-stride free dim on the u operand); False = 4 strided ops
FUSED_DVE = True


def _filter_matrix() -> np.ndarray:
    """A[h, m]: m in 0..127 -> even output row 2m; m in 128..255 -> odd row
    2(m-128)+1. Carries the full 1/64 scale of the separable pass."""
    A = np.zeros((H, 2 * H), np.float32)
    for i in range(H):
        # even output row 2i = (1*x[i-1] + 3*x[i])/64
        if i - 1 >= 0:
            A[i - 1, i] = 1.0 / 64
        A[i, i] = 3.0 / 64
        # odd output row 2i+1 = (3*x[i] + 1*x[i+1])/64
        A[i, H + i] = 3.0 / 64
        if i + 1 < H:
            A[i + 1, H + i] = 1.0 / 64
    return A


def filter_input() -> np.ndarray:
    A = _filter_matrix()
    if ROWPAIR_LOAD:
        # [64, 512]: partitions = row-pair index p; cols 0:256 = A[2p, :],
        # cols 256:512 = A[2p+1, :]  (both matmul operands keep base
        # partition 0 - matmul requires lhsT/rhs partition ranges to match)
        A = np.concatenate([A[0::2], A[1::2]], axis=1)
    return np.ascontiguousarray(A)


def build_kernel_body(tc, x, filt, out, nimg):
    """Emit the kernel IR. x [nimg,128,128] f32, filt [128,256] f32,
    out [nimg,256,256] f32."""
    from contextlib import ExitStack

    import concourse.mybir as mybir
    from concourse.ap import AP

    f32 = mybir.dt.float32
    f32r = mybir.dt.float32r
    nc = tc.nc
    ngroups = nimg // GROUP
    GW = GROUP * W  # 512

    LEAD = 6  # input loads issued this many groups ahead of use

    with ExitStack() as ctx:
        const_pool = ctx.enter_context(tc.tile_pool(name="const", bufs=1))
        xin_pool = ctx.enter_context(tc.tile_pool(name="xin", bufs=LEAD + 3))
        v_pool = ctx.enter_context(tc.tile_pool(name="v", bufs=4, space="PSUM"))
        o_pool = ctx.enter_context(tc.tile_pool(name="o", bufs=8))
        u_pool = ctx.enter_context(tc.tile_pool(name="u", bufs=4))

        A = const_pool.tile([64, 512] if ROWPAIR_LOAD else [128, 256], f32r)
        nc.scalar.dma_start(A[:], filt.bitcast(f32r))

        xg_tiles = {}

        def issue_load(gl):
            if gl >= ngroups:
                return
            j0 = gl * GROUP
            eng = nc.sync  # loads live alone on the SP ring -> deep prefetch
            if ROWPAIR_LOAD:
                xg = xin_pool.tile([64, 2 * GW], f32r)
                # partition p <- rows (2p, 2p+1): 1KB contiguous per (p, img)
                src = (
                    x[j0 : j0 + GROUP]
                    .rearrange("i (p r) w -> p i (r w)", r=2)
                    .bitcast(f32r)
                )
                dst = xg[:].rearrange("p (i rw) -> p i rw", i=GROUP)
            else:
                xg = xin_pool.tile([128, GW], f32r)
                src = x[j0 : j0 + GROUP].rearrange("i h w -> h i w").bitcast(f32r)
                dst = xg[:].rearrange("p (i w) -> p i w", i=GROUP)
            xg_tiles[gl] = xg
            eng.dma_start(dst, src)

        # startup burst: prefetch the first LEAD groups
        for gl in range(LEAD):
            issue_load(gl)

        for g in range(ngroups):
            issue_load(g + LEAD)
            i0 = g * GROUP
            xg = xg_tiles.pop(g)

            # pass 1 (vertical) on TensorE; partition p of v holds:
            #   cols 0:512   = V[2p,   (img, w)]   (even phase)
            #   cols 512:1024= V[2p+1, (img, w)]   (odd phase)
            v = v_pool.tile([128, 2 * GW], f32)
            if ROWPAIR_LOAD:
                xr = xg[:].rearrange("p (i r w) -> p r i w", r=2, i=GROUP)
                for eo in range(2):
                    m0 = eo * 128
                    nc.tensor.matmul(
                        v[:, eo * GW : (eo + 1) * GW],
                        A[:, m0 : m0 + 128],
                        xr[:, 0],
                        start=True,
                        stop=False,
                    )
                    nc.tensor.matmul(
                        v[:, eo * GW : (eo + 1) * GW],
                        A[:, 256 + m0 : 256 + m0 + 128],
                        xr[:, 1],
                        start=False,
                        stop=True,
                    )
            else:
                nc.tensor.matmul(v[:, 0:GW], A[:, 0:128], xg[:], start=True, stop=True)
                nc.tensor.matmul(
                    v[:, GW : 2 * GW], A[:, 128:256], xg[:], start=True, stop=True
                )

            # u = 3V on ScalarE (SBUF), so DVE reads one PSUM operand max
            u = u_pool.tile([128, 2 * GW], f32)
            nc.scalar.mul(u[:], v[:], 3.0)

            # out tile: partition p = output rows (2p, 2p+1):
            #   layout [img, eo, c] -> (c2 c) contiguous 2KB per (img)
            o = o_pool.tile([128, 2 * GROUP * 2 * W], f32)
            vE = v[:].rearrange("p (eo i w) -> p eo i w", eo=2, i=GROUP)
            vI = v[:].rearrange("p (eo i w) -> p i eo w", eo=2, i=GROUP)
            uE = u[:].rearrange("p (eo i w) -> p eo i w", eo=2, i=GROUP)
            uI = u[:].rearrange("p (eo i w) -> p i eo w", eo=2, i=GROUP)
            o4 = o[:].rearrange("p (i eo c) -> p i eo c", i=GROUP, eo=2)

            if FUSED_DVE:
                # one op per eo over (img, j=1..126, k=0..1):
                #   out[2j+k] = u[j] + V[j-1+2k]; contiguous writes
                ovv = o[:]
                uvv = u[:]
                vvv = v[:]
                pdim_o = list(ovv.ap[0])
                pdim_u = list(uvv.ap[0])
                pdim_v = list(vvv.ap[0])
                for eo in range(2):
                    out_ap = AP(
                        ovv.tensor,
                        ovv.offset + 256 * eo + 2,
                        [pdim_o, [512, GROUP], [2, 126], [1, 2]],
                    )
                    u_ap = AP(
                        uvv.tensor,
                        uvv.offset + 512 * eo + 1,
                        [pdim_u, [128, GROUP], [1, 126], [0, 2]],
                    )
                    v_ap = AP(
                        vvv.tensor,
                        vvv.offset + 512 * eo,
                        [pdim_v, [128, GROUP], [1, 126], [2, 2]],
                    )
                    nc.vector.tensor_add(out_ap, u_ap, v_ap)
            else:
                # interior: j = 1..126, col 2j+k = u[j] + V[j-1+2k]
                for eo in range(2):
                    for k in range(2):
                        nc.vector.tensor_add(
                            o4[:, :, eo, 2 + k : 254 + k : 2],
                            uE[:, eo, :, 1:127],
                            vE[:, eo, :, 2 * k : 2 * k + 126],
                        )
            # edge cols {1, 254}: 3*V[0]+V[1], 3*V[127]+V[126]
            nc.vector.tensor_add(
                o4[:, :, :, 1:255:253],
                uI[:, :, :, 0:128:127],
                vI[:, :, :, 1:127:125],
            )
            # seam cols {0, 255}: 3*V[0], 3*V[127] on ScalarE
            nc.scalar.copy(o4[:, :, :, 0:256:255], uI[:, :, :, 0:128:127])

            # one DMA for the whole group: partition p -> DRAM rows 2p, 2p+1
            # stores alternate between the two HWDGE rings (SP / ACT); on the
            # SP ring they sit AFTER the next input-load issue, so the LEAD
            # groups of slack keep prefetch from blocking behind them
            dst = out[i0 : i0 + GROUP].rearrange("i (p c2) c -> p i (c2 c)", c2=2)
            out_eng = nc.sync if g % 2 == 1 else nc.scalar
            out_eng.dma_start(dst, o[:].rearrange("p (i cc) -> p i cc", i=GROUP))


def build_bass(nimg=NIMG_PER_CORE, enable_asserts=False):
    import concourse.bacc as bacc
    import concourse.mybir as mybir
    import concourse.tile as tile

    f32 = mybir.dt.float32
    nc = bacc.Bacc(
        "TRN2",
        target_bir_lowering=False,
        debug=False,
        enable_asserts=enable_asserts,
        num_devices=N_CORES,
    )
    x = nc.dram_tensor("x", [nimg, H, W], f32, kind="ExternalInput").ap()
    fshape = [H // 2, 4 * H] if ROWPAIR_LOAD else [H, 2 * H]
    filt = nc.dram_tensor("filt", fshape, f32, kind="ExternalInput").ap()
    out = nc.dram_tensor("out", [nimg, 2 * H, 2 * W], f32, kind="ExternalOutput").ap()
    with tile.TileContext(nc) as tc:
        build_kernel_body(tc, x, filt, out, nimg)
    nc.compile()
    return nc


_NC_CACHE = {}


def kernel(x: np.ndarray, _trace=False, _trace_cores=None) -> np.ndarray:
    from concourse.bass_utils import run_bass_kernel_spmd

    assert x.shape == (16, 128, H, W), x.shape
    xf = np.ascontiguousarray(x, dtype=np.float32).reshape(N_CORES, NIMG_PER_CORE, H, W)
    A = filter_input()
    in_maps = [{"x": xf[k], "filt": A} for k in range(N_CORES)]

    key = NIMG_PER_CORE
    if key not in _NC_CACHE:
        _NC_CACHE[key] = build_bass()
    nc = _NC_CACHE[key]

    res = run_bass_kernel_spmd(
        nc,
        in_maps,
        core_ids=list(range(N_CORES)),
        trace=_trace,
        trace_cores=_trace_cores,
    )
    outs = np.stack([r["out"] for r in res.results])  # [8, 256, 256, 256]
    out = outs.reshape(16, 128, 2 * H, 2 * W)
    if _trace:
        kernel._last_result = res
    return out
